# revision 3
# baseline (speedup 1.0000x reference)
"""Trainium2 Bass kernel for ActSWL:
    X_t = cumsum_T(x), Y = silu(X), out_t = Y_t - Y_{t-1}  (Y_{-1}=0)

Input x: (T=4, B=2, L=2048, D=4096) f32. The recurrence is only over T and is
independent per (B,L,D) element, so we shard the flattened B*L*D axis across
8 NeuronCores (2Mi contiguous elements per core, viewed as [T, 128, 16384]).
"""

import sys

sys.path.insert(0, "/opt/trn_rl_repo")

import numpy as np

import concourse.bass as bass
import concourse.tile as tile
from concourse import bacc, mybir
from concourse.bass_utils import run_bass_kernel_spmd

T, B, L, D = 4, 2, 2048, 4096
N_CORES = 8
M = B * L * D                     # 16_777_216 flattened per-t elements
PER_CORE = M // N_CORES           # 2_097_152
P = 128                           # SBUF partitions
FREE = PER_CORE // P              # 16384 f32 per partition per t
F = 2048                          # free-dim chunk size (1 MiB per-t DMA)
N_CHUNKS = FREE // F

_cache = {}


def _build_nc(f=F, bufs=2, iters=1):
    fp32 = mybir.dt.float32
    silu = mybir.ActivationFunctionType.Silu
    n_chunks = FREE // f

    nc = bacc.Bacc("TRN2", debug=False, num_devices=N_CORES)
    x_d = nc.dram_tensor("x", [T, P, FREE], fp32, kind="ExternalInput").ap()
    o_d = nc.dram_tensor("out", [T, P, FREE], fp32, kind="ExternalOutput").ap()

    with tile.TileContext(nc) as tc:
        with (
            tc.tile_pool(name="xin", bufs=bufs) as xin_pool,
            tc.tile_pool(name="oot", bufs=bufs) as oot_pool,
            tc.tile_pool(name="ys", bufs=bufs) as y_pool,
        ):

            def body():
                for c in range(n_chunks):
                    xin = xin_pool.tile([P, T * f], fp32, tag="xin")
                    oot = oot_pool.tile([P, T * f], fp32, tag="oot")
                    y1 = y_pool.tile([P, f], fp32, tag="y1")
                    y2 = y_pool.tile([P, f], fp32, tag="y2")

                    I = [xin[:, bass.ts(t, f)] for t in range(T)]
                    O = [oot[:, bass.ts(t, f)] for t in range(T)]
                    for t in range(T):
                        nc.sync.dma_start(out=I[t], in_=x_d[t, :, bass.ts(c, f)])

                    # t=0: out0 = Y0 = silu(x0); O[0] doubles as Y0 storage
                    nc.scalar.activation(O[0], I[0], silu)
                    # t=1: X1 = X0 + x1 (into I[1]); Y1 = silu(X1); out1 = Y1-Y0
                    nc.vector.tensor_add(I[1], I[0], I[1])
                    nc.scalar.activation(y1[:], I[1], silu)
                    nc.vector.tensor_sub(O[1], y1[:], O[0])
                    # t=2
                    nc.vector.tensor_add(I[2], I[1], I[2])
                    nc.scalar.activation(y2[:], I[2], silu)
                    nc.vector.tensor_sub(O[2], y2[:], y1[:])
                    # t=3: Y3 computed in place into I[3]
                    nc.vector.tensor_add(I[3], I[2], I[3])
                    nc.scalar.activation(I[3], I[3], silu)
                    nc.vector.tensor_sub(O[3], I[3], y2[:])

                    for t in range(T):
                        nc.sync.dma_start(out=o_d[t, :, bass.ts(c, f)], in_=O[t])

            if iters == 1:
                body()
            else:
                with tc.For_i(0, iters, 1):
                    body()

    nc.compile()
    return nc


def _get_nc(**kw):
    key = tuple(sorted(kw.items()))
    if key not in _cache:
        _cache[key] = _build_nc(**kw)
    return _cache[key]


def run(x: np.ndarray, trace: bool = False, **build_kw):
    """Shard, execute on 8 cores, gather. Returns (out, BassKernelResults)."""
    nc = _get_nc(**build_kw)
    xf = np.ascontiguousarray(x, dtype=np.float32).reshape(T, M)
    in_maps = []
    for i in range(N_CORES):
        shard = np.ascontiguousarray(
            xf[:, i * PER_CORE : (i + 1) * PER_CORE]
        ).reshape(T, P, FREE)
        in_maps.append({"x": shard})
    res = run_bass_kernel_spmd(
        nc, in_maps, core_ids=list(range(N_CORES)), trace=trace
    )
    out = np.empty((T, M), dtype=np.float32)
    for i in range(N_CORES):
        out[:, i * PER_CORE : (i + 1) * PER_CORE] = res.results[i]["out"].reshape(
            T, PER_CORE
        )
    return out.reshape(T, B, L, D), res


def kernel(x: np.ndarray) -> np.ndarray:
    out, _ = run(x)
    return out


class Runner:
    """Persistent-jit executor mirroring bass2jax.run_bass_via_pjrt's
    multi-core path, but caching the jitted callable and device-resident
    inputs so repeated calls measure steady-state device execution."""

    def __init__(self, nc, n_cores=N_CORES):
        import jax
        from jax.sharding import Mesh, PartitionSpec, NamedSharding
        from jax.experimental.shard_map import shard_map
        from concourse import bass2jax

        bass2jax.install_neuronx_cc_hook()
        self.jax = jax
        partition_name = (
            nc.partition_id_tensor.name if nc.partition_id_tensor else None
        )
        in_names, out_names, out_avals, zero_outs = [], [], [], []
        for alloc in nc.m.functions[0].allocations:
            if not isinstance(alloc, mybir.MemoryLocationSet):
                continue
            name = alloc.memorylocations[0].name
            if alloc.kind == "ExternalInput":
                if name != partition_name:
                    in_names.append(name)
            elif alloc.kind == "ExternalOutput":
                shape = tuple(alloc.tensor_shape)
                dtype = mybir.dt.np(alloc.dtype)
                out_names.append(name)
                out_avals.append(jax.core.ShapedArray(shape, dtype))
                zero_outs.append((shape, dtype))
        n_params = len(in_names)
        n_outs = len(out_avals)
        in_names_ext = list(in_names) + list(out_names)
        if partition_name is not None:
            in_names_ext.append(partition_name)
        donate = tuple(range(n_params, n_params + n_outs))

        def _body(*args):
            operands = list(args)
            if partition_name is not None:
                operands.append(bass2jax.partition_id_tensor())
            outs = bass2jax._bass_exec_p.bind(
                *operands,
                out_avals=tuple(out_avals),
                in_names=tuple(in_names_ext),
                out_names=tuple(out_names),
                lowering_input_output_aliases=(),
                sim_require_finite=True,
                sim_require_nnan=True,
                nc=nc,
            )
            return tuple(outs)

        devices = jax.devices()[:n_cores]
        mesh = Mesh(np.asarray(devices), ("core",))
        in_specs = (PartitionSpec("core"),) * (n_params + n_outs)
        out_specs = (PartitionSpec("core"),) * n_outs
        self.fn = jax.jit(
            shard_map(
                _body,
                mesh=mesh,
                in_specs=in_specs,
                out_specs=out_specs,
                check_rep=False,
            ),
            donate_argnums=donate,
            keep_unused=True,
        )
        self.sharding = NamedSharding(mesh, PartitionSpec("core"))
        import jax.numpy as jnp

        def _zeros():
            return tuple(
                jnp.zeros((n_cores * s[0], *s[1:]), d) for s, d in zero_outs
            )

        self.zeros_fn = jax.jit(
            _zeros, out_shardings=(self.sharding,) * n_outs
        )
        self.in_names = in_names
        self.out_names = out_names
        self.out_avals = out_avals
        self.n_cores = n_cores

    def put_inputs(self, in_maps):
        concat = [
            np.concatenate([np.asarray(m[k]) for m in in_maps], axis=0)
            for k in self.in_names
        ]
        return [self.jax.device_put(a, self.sharding) for a in concat]

    def __call__(self, in_dev):
        zs = self.zeros_fn()
        outs = self.fn(*in_dev, *zs)
        return outs

    def timeit(self, in_dev, warmup=2, reps=10):
        import time as _t

        for _ in range(warmup):
            o = self(in_dev)
            self.jax.block_until_ready(o)
        times = []
        for _ in range(reps):
            zs = self.zeros_fn()
            self.jax.block_until_ready(zs)
            t0 = _t.perf_counter()
            o = self.fn(*in_dev, *zs)
            self.jax.block_until_ready(o)
            times.append(_t.perf_counter() - t0)
        return times


# revision 11
# speedup vs baseline: 1.0043x; 1.0043x over previous
"""Trainium2 Bass kernel for ActSWL:
    X_t = cumsum_T(x), Y = silu(X), out_t = Y_t - Y_{t-1}  (Y_{-1}=0)

Input x: (T=4, B=2, L=2048, D=4096) f32. The recurrence is only over T and is
independent per (B,L,D) element, so we shard the flattened B*L*D axis across
8 NeuronCores (2Mi contiguous elements per core, viewed as [T, 128, 16384]).
"""

import sys

sys.path.insert(0, "/opt/trn_rl_repo")

import numpy as np

import concourse.bass as bass
import concourse.tile as tile
from concourse import bacc, mybir
from concourse.bass_utils import run_bass_kernel_spmd

T, B, L, D = 4, 2, 2048, 4096
N_CORES = 8
M = B * L * D                     # 16_777_216 flattened per-t elements
PER_CORE = M // N_CORES           # 2_097_152
P = 128                           # SBUF partitions
FREE = PER_CORE // P              # 16384 f32 per partition per t
F = 2048                          # free-dim chunk size (1 MiB per-t DMA)
N_CHUNKS = FREE // F

_cache = {}


def _chunk_list(f, taper):
    if not taper:
        return [f] * (FREE // f)
    head = [f // 4, f // 4, f // 2]
    mid = [f] * ((FREE - 2 * sum(head)) // f)
    assert sum(head) * 2 + sum(mid) == FREE
    return head + mid + head[::-1]


def _build_nc(
    f=F,
    bufs=2,
    iters=1,
    store_eng="sync",
    fused=False,
    taper=False,
    inplace=False,
):
    fp32 = mybir.dt.float32
    silu = mybir.ActivationFunctionType.Silu
    chunks = _chunk_list(f, taper)

    nc = bacc.Bacc("TRN2", debug=False, num_devices=N_CORES)
    store = getattr(nc, store_eng)
    x_d = nc.dram_tensor("x", [T, P, FREE], fp32, kind="ExternalInput").ap()
    o_d = nc.dram_tensor("out", [T, P, FREE], fp32, kind="ExternalOutput").ap()

    with tile.TileContext(nc) as tc:
        with (
            tc.tile_pool(name="xin", bufs=bufs) as xin_pool,
            tc.tile_pool(name="oot", bufs=bufs) as oot_pool,
            tc.tile_pool(name="ys", bufs=bufs) as y_pool,
        ):

            def chunk_inplace2(fc, sl):
                # 40KB/partition per chunk: input tile + ONE temp; out2
                # reuses I1's slot, out3 reuses I3's.
                xin = xin_pool.tile([P, T, f], fp32, tag="xin")
                y1 = y_pool.tile([P, f], fp32, tag="y1")
                I = [xin[:, t, 0:fc] for t in range(T)]
                Y1 = y1[:, 0:fc]
                if fused:
                    nc.sync.dma_start(
                        out=xin[:, :, 0:fc],
                        in_=x_d[:, :, sl].transpose([1, 0, 2]),
                    )
                else:
                    for t in range(T):
                        nc.sync.dma_start(out=I[t], in_=x_d[t, :, sl])

                nc.vector.tensor_add(I[1], I[0], I[1])      # X1
                nc.scalar.activation(I[0], I[0], silu)      # I0 <- Y0 = out0
                store.dma_start(out=o_d[0, :, sl], in_=I[0])
                nc.vector.tensor_add(I[2], I[1], I[2])      # X2
                nc.scalar.activation(I[1], I[1], silu)      # I1 <- Y1
                nc.vector.tensor_sub(Y1, I[1], I[0])        # y1 <- out1
                store.dma_start(out=o_d[1, :, sl], in_=Y1)
                nc.vector.tensor_add(I[3], I[2], I[3])      # X3
                nc.scalar.activation(I[2], I[2], silu)      # I2 <- Y2
                nc.vector.tensor_sub(I[1], I[2], I[1])      # I1 <- out2
                store.dma_start(out=o_d[2, :, sl], in_=I[1])
                nc.scalar.activation(I[3], I[3], silu)      # I3 <- Y3
                nc.vector.tensor_sub(I[3], I[3], I[2])      # I3 <- out3
                store.dma_start(out=o_d[3, :, sl], in_=I[3])

            def chunk_inplace(fc, sl):
                # outputs computed into the input tile + 2 small temps:
                # 48KB/partition per chunk instead of 80KB.
                xin = xin_pool.tile([P, T, f], fp32, tag="xin")
                y1 = y_pool.tile([P, f], fp32, tag="y1")
                y2 = y_pool.tile([P, f], fp32, tag="y2")
                I = [xin[:, t, 0:fc] for t in range(T)]
                Y1, Y2 = y1[:, 0:fc], y2[:, 0:fc]
                if fused:
                    nc.sync.dma_start(
                        out=xin[:, :, 0:fc],
                        in_=x_d[:, :, sl].transpose([1, 0, 2]),
                    )
                else:
                    for t in range(T):
                        nc.sync.dma_start(out=I[t], in_=x_d[t, :, sl])

                nc.vector.tensor_add(I[1], I[0], I[1])      # X1
                nc.scalar.activation(I[0], I[0], silu)      # I0 <- Y0 = out0
                store.dma_start(out=o_d[0, :, sl], in_=I[0])
                nc.vector.tensor_add(I[2], I[1], I[2])      # X2
                nc.scalar.activation(I[1], I[1], silu)      # I1 <- Y1
                nc.vector.tensor_sub(Y1, I[1], I[0])        # y1 <- out1
                store.dma_start(out=o_d[1, :, sl], in_=Y1)
                nc.vector.tensor_add(I[3], I[2], I[3])      # X3
                nc.scalar.activation(I[2], I[2], silu)      # I2 <- Y2
                nc.vector.tensor_sub(Y2, I[2], I[1])        # y2 <- out2
                store.dma_start(out=o_d[2, :, sl], in_=Y2)
                nc.scalar.activation(I[3], I[3], silu)      # I3 <- Y3
                nc.vector.tensor_sub(I[3], I[3], I[2])      # I3 <- out3
                store.dma_start(out=o_d[3, :, sl], in_=I[3])

            def chunk_sep(fc, sl):
                xin = xin_pool.tile([P, T, f], fp32, tag="xin")
                oot = oot_pool.tile([P, T, f], fp32, tag="oot")
                y1 = y_pool.tile([P, f], fp32, tag="y1")
                y2 = y_pool.tile([P, f], fp32, tag="y2")

                I = [xin[:, t, 0:fc] for t in range(T)]
                O = [oot[:, t, 0:fc] for t in range(T)]
                if fused:
                    nc.sync.dma_start(
                        out=xin[:, :, 0:fc],
                        in_=x_d[:, :, sl].transpose([1, 0, 2]),
                    )
                else:
                    for t in range(T):
                        nc.sync.dma_start(out=I[t], in_=x_d[t, :, sl])

                # t=0: out0 = Y0 = silu(x0); O[0] doubles as Y0 storage
                nc.scalar.activation(O[0], I[0], silu)
                # t=1: X1 = X0 + x1 (into I[1]); Y1 = silu(X1); out1 = Y1-Y0
                nc.vector.tensor_add(I[1], I[0], I[1])
                nc.scalar.activation(y1[:, 0:fc], I[1], silu)
                nc.vector.tensor_sub(O[1], y1[:, 0:fc], O[0])
                # t=2
                nc.vector.tensor_add(I[2], I[1], I[2])
                nc.scalar.activation(y2[:, 0:fc], I[2], silu)
                nc.vector.tensor_sub(O[2], y2[:, 0:fc], y1[:, 0:fc])
                # t=3: Y3 computed in place into I[3]
                nc.vector.tensor_add(I[3], I[2], I[3])
                nc.scalar.activation(I[3], I[3], silu)
                nc.vector.tensor_sub(O[3], I[3], y2[:, 0:fc])

                if fused:
                    store.dma_start(
                        out=o_d[:, :, sl].transpose([1, 0, 2]),
                        in_=oot[:, :, 0:fc],
                    )
                else:
                    for t in range(T):
                        store.dma_start(out=o_d[t, :, sl], in_=O[t])

            def body():
                off = 0
                for fc in chunks:
                    sl = slice(off, off + fc)
                    if inplace == 2:
                        chunk_inplace2(fc, sl)
                    elif inplace:
                        chunk_inplace(fc, sl)
                    else:
                        chunk_sep(fc, sl)
                    off += fc

            if iters == 1:
                body()
            else:
                with tc.For_i(0, iters, 1):
                    body()

    nc.compile()
    return nc


def _get_nc(**kw):
    key = tuple(sorted(kw.items()))
    if key not in _cache:
        _cache[key] = _build_nc(**kw)
    return _cache[key]


BEST = dict(f=2048, bufs=3, store_eng="gpsimd", inplace=True)


def run(x: np.ndarray, trace: bool = False, **build_kw):
    """Shard, execute on 8 cores, gather. Returns (out, BassKernelResults)."""
    nc = _get_nc(**{**BEST, **build_kw})
    xf = np.ascontiguousarray(x, dtype=np.float32).reshape(T, M)
    in_maps = []
    for i in range(N_CORES):
        shard = np.ascontiguousarray(
            xf[:, i * PER_CORE : (i + 1) * PER_CORE]
        ).reshape(T, P, FREE)
        in_maps.append({"x": shard})
    res = run_bass_kernel_spmd(
        nc, in_maps, core_ids=list(range(N_CORES)), trace=trace
    )
    out = np.empty((T, M), dtype=np.float32)
    for i in range(N_CORES):
        out[:, i * PER_CORE : (i + 1) * PER_CORE] = res.results[i]["out"].reshape(
            T, PER_CORE
        )
    return out.reshape(T, B, L, D), res


def kernel(x: np.ndarray) -> np.ndarray:
    out, _ = run(x)
    return out


class Runner:
    """Persistent-jit executor mirroring bass2jax.run_bass_via_pjrt's
    multi-core path, but caching the jitted callable and device-resident
    inputs so repeated calls measure steady-state device execution."""

    def __init__(self, nc, n_cores=N_CORES):
        import jax
        from jax.sharding import Mesh, PartitionSpec, NamedSharding
        from jax.experimental.shard_map import shard_map
        from concourse import bass2jax

        bass2jax.install_neuronx_cc_hook()
        self.jax = jax
        partition_name = (
            nc.partition_id_tensor.name if nc.partition_id_tensor else None
        )
        in_names, out_names, out_avals, zero_outs = [], [], [], []
        for alloc in nc.m.functions[0].allocations:
            if not isinstance(alloc, mybir.MemoryLocationSet):
                continue
            name = alloc.memorylocations[0].name
            if alloc.kind == "ExternalInput":
                if name != partition_name:
                    in_names.append(name)
            elif alloc.kind == "ExternalOutput":
                shape = tuple(alloc.tensor_shape)
                dtype = mybir.dt.np(alloc.dtype)
                out_names.append(name)
                out_avals.append(jax.core.ShapedArray(shape, dtype))
                zero_outs.append((shape, dtype))
        n_params = len(in_names)
        n_outs = len(out_avals)
        in_names_ext = list(in_names) + list(out_names)
        if partition_name is not None:
            in_names_ext.append(partition_name)
        donate = tuple(range(n_params, n_params + n_outs))

        def _body(*args):
            operands = list(args)
            if partition_name is not None:
                operands.append(bass2jax.partition_id_tensor())
            outs = bass2jax._bass_exec_p.bind(
                *operands,
                out_avals=tuple(out_avals),
                in_names=tuple(in_names_ext),
                out_names=tuple(out_names),
                lowering_input_output_aliases=(),
                sim_require_finite=True,
                sim_require_nnan=True,
                nc=nc,
            )
            return tuple(outs)

        devices = jax.devices()[:n_cores]
        mesh = Mesh(np.asarray(devices), ("core",))
        in_specs = (PartitionSpec("core"),) * (n_params + n_outs)
        out_specs = (PartitionSpec("core"),) * n_outs
        self.fn = jax.jit(
            shard_map(
                _body,
                mesh=mesh,
                in_specs=in_specs,
                out_specs=out_specs,
                check_rep=False,
            ),
            donate_argnums=donate,
            keep_unused=True,
        )
        self.sharding = NamedSharding(mesh, PartitionSpec("core"))
        import jax.numpy as jnp

        def _zeros():
            return tuple(
                jnp.zeros((n_cores * s[0], *s[1:]), d) for s, d in zero_outs
            )

        self.zeros_fn = jax.jit(
            _zeros, out_shardings=(self.sharding,) * n_outs
        )
        self.in_names = in_names
        self.out_names = out_names
        self.out_avals = out_avals
        self.n_cores = n_cores

    def put_inputs(self, in_maps):
        concat = [
            np.concatenate([np.asarray(m[k]) for m in in_maps], axis=0)
            for k in self.in_names
        ]
        return [self.jax.device_put(a, self.sharding) for a in concat]

    def __call__(self, in_dev):
        zs = self.zeros_fn()
        outs = self.fn(*in_dev, *zs)
        return outs

    def timeit(self, in_dev, warmup=2, reps=10):
        import time as _t

        for _ in range(warmup):
            o = self(in_dev)
            self.jax.block_until_ready(o)
        times = []
        for _ in range(reps):
            zs = self.zeros_fn()
            self.jax.block_until_ready(zs)
            t0 = _t.perf_counter()
            o = self.fn(*in_dev, *zs)
            self.jax.block_until_ready(o)
            times.append(_t.perf_counter() - t0)
        return times
